# revision 19
# baseline (speedup 1.0000x reference)
"""Masked attention (B=16, S=1024, H=1024) on 8 TRN2 NeuronCores.

Strategy: pure data-parallel over batch — 2 batches per core, no collectives.

Sparsity: the mask zeroes ~half of the key columns per batch, and masked
columns contribute exactly-zero attention weights (exp(-1e9 - max) underflows
to 0 in f32, matching the reference bit-for-bit).  So the kernel gathers the
unmasked columns on the host, runs attention over a compact key/value axis of
UP columns (max unmasked count over batches, rounded up to 128), and the host
scatters the compact weight matrix back into the dense [S, S] output (masked
columns stay 0).  If any batch has zero unmasked columns the kernel falls
back to the dense path (UP = S, identity gather) through the same graph.

Per batch (X = input[b] [S, H], XU = unmasked-gathered columns [UP, H]):
  qT  = (Wq/sqrt(H)) @ X.T + bq/sqrt(H)   -> [H, S]
  kTU = Wk @ XU.T + bk                    -> [H, UP]
  vU  = XU @ Wv.T + bv                    -> [UP, H]
  s   = qT.T @ kTU + padbias              -> [S, UP]  (padbias = -1e9 on pad cols)
  e   = exp(s - rowmax); attwc = e / rowsum (compact weights out)
  att = (eT.T @ vU) / rowsum              -> e transposed on the PE

All TensorEngine operands are bf16 (pre-cast on host), accumulation f32 in
PSUM, softmax statistics f32.  End-to-end rel err vs the f32 reference ~5e-3.
"""
import numpy as np
import ml_dtypes

import concourse.bass as bass
import concourse.mybir as mybir
from concourse import bacc
from concourse.tile import TileContext
from concourse.bass_utils import run_bass_kernel_spmd
from concourse.masks import make_identity

B, S, H = 16, 1024, 1024
P = 128
NCORES = 8
B_LOC = B // NCORES          # batches per core
KT = H // P                  # 8 contraction tiles
RT = S // P                  # 8 query row blocks
NFREE = 512                  # matmul moving free dim (one PSUM bank)
BF16 = mybir.dt.bfloat16
F32 = mybir.dt.float32

_BUILD_CACHE = {}


def _chunks(total, step=NFREE):
    out = []
    o = 0
    while o < total:
        out.append((o, min(step, total - o)))
        o += step
    return out


def build(UP):
    """Build the SPMD graph for a compact key axis of UP columns."""
    if UP in _BUILD_CACHE:
        return _BUILD_CACHE[UP]
    assert UP % P == 0
    UT = UP // P
    UCH = _chunks(UP)
    SCH = _chunks(S)
    HCH = _chunks(H)

    nc = bacc.Bacc()

    # All inputs arrive pre-tiled in SBUF layout (partition-major, contiguous
    # per partition) so DMA bursts are kilobytes, not 256B strided runs.
    xT = nc.declare_dram_parameter("xT", [B_LOC, P, KT, S], BF16, isOutput=False)
    xTU = nc.declare_dram_parameter("xTU", [B_LOC, P, KT, UP], BF16, isOutput=False)
    wq0 = nc.declare_dram_parameter("wq0", [P, KT, P], BF16, isOutput=False)
    wqr = nc.declare_dram_parameter("wqr", [P, KT, H - P], BF16, isOutput=False)
    wkf = nc.declare_dram_parameter("wkf", [P, KT, H], BF16, isOutput=False)
    wvf = nc.declare_dram_parameter("wvf", [P, KT, H], BF16, isOutput=False)
    bqp = nc.declare_dram_parameter("bqp", [P, KT], F32, isOutput=False)
    bkp = nc.declare_dram_parameter("bkp", [P, KT], F32, isOutput=False)
    bvr = nc.declare_dram_parameter("bvr", [P, H], F32, isOutput=False)
    mkb = nc.declare_dram_parameter("mkb", [B_LOC, P, UP], F32, isOutput=False)
    att = nc.declare_dram_parameter("att", [B_LOC, S, H], F32, isOutput=True)
    attwc = nc.declare_dram_parameter("attwc", [B_LOC, S, UP], F32, isOutput=True)

    with TileContext(nc) as tc:
        with (
            tc.tile_pool(name="const", bufs=1) as constp,
            tc.tile_pool(name="wpool", bufs=1) as wpool,
            tc.tile_pool(name="xpool", bufs=2) as xpool,
            tc.tile_pool(name="qkv", bufs=1) as qkvp,
            tc.tile_pool(name="soft", bufs=2) as soft,
            tc.tile_pool(name="stats", bufs=3) as stats,
            tc.tile_pool(name="psmm", bufs=6, space="PSUM") as psmm,
            tc.tile_pool(name="pstr", bufs=2, space="PSUM") as pstr,
        ):
            ident = constp.tile([P, P], BF16)
            make_identity(nc, ident)
            bq_t = constp.tile([P, KT], F32)
            bk_t = constp.tile([P, KT], F32)
            bv_t = constp.tile([P, H], F32)
            nc.gpsimd.dma_start(out=bq_t, in_=bqp[:, :])
            nc.gpsimd.dma_start(out=bk_t, in_=bkp[:, :])
            nc.gpsimd.dma_start(out=bv_t, in_=bvr[:, :])

            # DMA issue is serialized per engine (~2.7us per big transfer on
            # the issuing sequencer), so spread it: weights issue on the
            # scalar engine's HWDGE concurrently with x-batches on sync's.
            wq_t = wpool.tile([P, KT, H], BF16)
            wk_t = wpool.tile([P, KT, H], BF16)
            wv_t = wpool.tile([P, KT, H], BF16)
            xT0_t = xpool.tile([P, KT, S], BF16, name="xT0_t", tag="xT")
            xTU0_t = xpool.tile([P, KT, UP], BF16, name="xTU0_t", tag="xTU")
            nc.scalar.dma_start(out=wq_t[:, :, 0:P], in_=wq0[:])
            for off, csz in SCH:
                nc.sync.dma_start(out=xT0_t[:, :, off:off + csz], in_=xT[0][:, :, off:off + csz])
            nc.scalar.dma_start(out=wq_t[:, :, P:], in_=wqr[:])
            nc.sync.dma_start(out=xTU0_t, in_=xTU[0])
            nc.scalar.dma_start(out=wk_t, in_=wkf[:])
            nc.scalar.dma_start(out=wv_t, in_=wvf[:])

            def load_batch_inputs(b):
                if b == 0:
                    xb_t, xbU_t = xT0_t, xTU0_t
                else:
                    xb_t = xpool.tile([P, KT, S], BF16, name="xT_t", tag="xT")
                    nc.sync.dma_start(out=xb_t, in_=xT[b])
                    xbU_t = xpool.tile([P, KT, UP], BF16, name="xTU_t", tag="xTU")
                    nc.sync.dma_start(out=xbU_t, in_=xTU[b])
                mkb_t = xpool.tile([P, UP], F32, name="mkb_t", tag="mkb")
                nc.sync.dma_start(out=mkb_t, in_=mkb[b])
                return xb_t, xbU_t, mkb_t

            next_inputs = load_batch_inputs(0)
            for b in range(B_LOC):
                xT_t, xTU_t, mkb_t = next_inputs

                qT_t = qkvp.tile([P, KT, S], BF16, name="qT_t", tag="qT")
                kT_t = qkvp.tile([P, KT, UP], BF16, name="kT_t", tag="kT")
                v_t = qkvp.tile([P, UT, H], BF16, name="v_t", tag="v")

                # ---- projections ----
                # qT[o, s] (full S) and kTU[o, u] (compact)
                for ot in range(KT):
                    for off, csz in SCH:
                        sl = slice(off, off + csz)
                        ps_q = psmm.tile([P, NFREE], F32, name="ps_q", tag="mm")[:, :csz]
                        for kt in range(KT):
                            nc.tensor.matmul(ps_q, wq_t[:, kt, ot * P:(ot + 1) * P],
                                             xT_t[:, kt, sl], start=(kt == 0), stop=(kt == KT - 1))
                        nc.vector.tensor_scalar_add(qT_t[:, ot, sl], ps_q, bq_t[:, ot:ot + 1])
                    for off, csz in UCH:
                        sl = slice(off, off + csz)
                        ps_k = psmm.tile([P, NFREE], F32, name="ps_k", tag="mm")[:, :csz]
                        for kt in range(KT):
                            nc.tensor.matmul(ps_k, wk_t[:, kt, ot * P:(ot + 1) * P],
                                             xTU_t[:, kt, sl], start=(kt == 0), stop=(kt == KT - 1))
                        nc.vector.tensor_scalar_add(kT_t[:, ot, sl], ps_k, bk_t[:, ot:ot + 1])
                # vU[u, o]
                for ut in range(UT):
                    for off, csz in HCH:
                        sl = slice(off, off + csz)
                        ps_v = psmm.tile([P, NFREE], F32, name="ps_v", tag="mm")[:, :csz]
                        for kt in range(KT):
                            nc.tensor.matmul(ps_v, xTU_t[:, kt, ut * P:(ut + 1) * P],
                                             wv_t[:, kt, sl], start=(kt == 0), stop=(kt == KT - 1))
                        nc.vector.tensor_tensor(out=v_t[:, ut, sl], in0=ps_v, in1=bv_t[:, sl],
                                                op=mybir.AluOpType.add)

                # Prefetch next batch's inputs now, so their sync-queue DMAs
                # sit ahead of this batch's output DMAs in the engine stream.
                if b + 1 < B_LOC:
                    next_inputs = load_batch_inputs(b + 1)

                # ---- attention, software-pipelined over row blocks ----
                # Emit scores+softmax for block r, then the PV half of block
                # r-1: the PE's static stream then runs scores(r+1) while the
                # DVE/ACT softmax of block r is still in flight, instead of
                # stalling ~3.5us per block waiting for e(r).
                def emit_scores_softmax(r):
                    sc_t = soft.tile([P, UP], F32, name="sc_t", tag="sc")
                    for off, csz in UCH:
                        sl = slice(off, off + csz)
                        ps_s = psmm.tile([P, NFREE], F32, name="ps_s", tag="mm")[:, :csz]
                        for ot in range(KT):
                            nc.tensor.matmul(ps_s, qT_t[:, ot, r * P:(r + 1) * P],
                                             kT_t[:, ot, sl], start=(ot == 0), stop=(ot == KT - 1))
                        nc.vector.tensor_tensor(out=sc_t[:, sl], in0=ps_s, in1=mkb_t[:, sl],
                                                op=mybir.AluOpType.add)

                    negmax = stats.tile([P, 1], F32, name="negmax", tag="negmax")
                    nc.vector.reduce_max(out=negmax, in_=sc_t, axis=mybir.AxisListType.X, negate=True)
                    e_t = soft.tile([P, UP], BF16, name="e_t", tag="e")
                    rowsum = stats.tile([P, 1], F32, name="rowsum", tag="rowsum")
                    nc.scalar.activation(out=e_t, in_=sc_t, func=mybir.ActivationFunctionType.Exp,
                                         bias=negmax, scale=1.0, accum_out=rowsum)
                    recip = stats.tile([P, 1], F32, name="recip", tag="recip")
                    nc.vector.reciprocal(out=recip, in_=rowsum)

                    # compact attention-weights output: attwc = e * recip
                    p_t = soft.tile([P, UP], F32, name="p_t", tag="p")
                    nc.vector.tensor_scalar_mul(p_t, e_t, recip)
                    nc.scalar.dma_start(out=attwc[b, r * P:(r + 1) * P, :], in_=p_t)
                    return e_t, recip

                def emit_pv(r, e_t, recip):
                    # transpose e on the PE: eT[u, i] tiles
                    eT_t = soft.tile([P, UT, P], BF16, name="eT_t", tag="eT")
                    for ut in range(UT):
                        ps_t = pstr.tile([P, P], BF16, name="ps_t", tag="tr")
                        nc.tensor.transpose(ps_t, e_t[:, ut * P:(ut + 1) * P], ident)
                        nc.scalar.activation(out=eT_t[:, ut], in_=ps_t,
                                             func=mybir.ActivationFunctionType.Copy)

                    # att[i, h] = sum_u e[i, u] v[u, h], normalized by recip
                    at_t = soft.tile([P, H], F32, name="at_t", tag="at")
                    for off, csz in HCH:
                        sl = slice(off, off + csz)
                        ps_a = psmm.tile([P, NFREE], F32, name="ps_a", tag="mm")[:, :csz]
                        for ut in range(UT):
                            nc.tensor.matmul(ps_a, eT_t[:, ut], v_t[:, ut, sl],
                                             start=(ut == 0), stop=(ut == UT - 1))
                        nc.vector.tensor_scalar_mul(at_t[:, sl], ps_a, recip)
                    nc.sync.dma_start(out=att[b, r * P:(r + 1) * P, :], in_=at_t)

                prev = None
                for r in range(RT):
                    cur = emit_scores_softmax(r)
                    if prev is not None:
                        emit_pv(r - 1, *prev)
                    prev = cur
                emit_pv(RT - 1, *prev)

    nc.finalize()
    _BUILD_CACHE[UP] = nc
    return nc


def _bf16(x):
    return np.ascontiguousarray(x.astype(ml_dtypes.bfloat16))


def kernel(input, mask, Wq, bq, Wk, bk, Wv, bv):
    input = np.asarray(input, dtype=np.float32)
    mask = np.asarray(mask)
    scale = np.float32(1.0 / np.sqrt(H))

    # Pre-tile weights into partition-major SBUF layout [p, t, o].
    wq_pm = _bf16(np.asarray(Wq).T * scale).reshape(KT, P, H).transpose(1, 0, 2)
    wq0 = np.ascontiguousarray(wq_pm[:, :, :P])
    wqr = np.ascontiguousarray(wq_pm[:, :, P:])
    wkf = np.ascontiguousarray(_bf16(np.asarray(Wk).T).reshape(KT, P, H).transpose(1, 0, 2))
    wvf = np.ascontiguousarray(_bf16(np.asarray(Wv).T).reshape(KT, P, H).transpose(1, 0, 2))
    bqp = np.ascontiguousarray((np.asarray(bq, dtype=np.float32) * scale).reshape(KT, P).T)
    bkp = np.ascontiguousarray(np.asarray(bk, dtype=np.float32).reshape(KT, P).T)
    bvr = np.ascontiguousarray(np.broadcast_to(np.asarray(bv, dtype=np.float32), (P, H)))

    m = np.asarray(mask[:, 0, 0, :])                     # [B, S]
    idxs = [np.nonzero(m[b] != 0)[0] for b in range(B)]
    ucounts = [len(ix) for ix in idxs]
    sparse = min(ucounts) > 0 and max(ucounts) < S
    if sparse:
        UP = max(P, ((max(ucounts) + P - 1) // P) * P)
    else:
        UP = S
        idxs = [np.arange(S) for _ in range(B)]
        ucounts = [S] * B

    in_maps = []
    for c in range(NCORES):
        xb = input[c * B_LOC:(c + 1) * B_LOC]            # [B_LOC, S, H]
        xTf = _bf16(xb.transpose(0, 2, 1))               # [B_LOC, H, S]
        xTU = np.zeros((B_LOC, H, UP), dtype=ml_dtypes.bfloat16)
        mkb = np.zeros((B_LOC, P, UP), dtype=np.float32)
        for bl in range(B_LOC):
            gb = c * B_LOC + bl
            u = ucounts[gb]
            xTU[bl, :, :u] = xTf[bl][:, idxs[gb]]
            if sparse:
                mkb[bl, :, u:] = np.float32(-1e9)
            else:
                mkb[bl, :, :] = np.where(m[gb] == 0, np.float32(-1e9), np.float32(0.0))[None, :]
        # pre-tile into partition-major SBUF layout
        xT_t = np.ascontiguousarray(
            xTf.reshape(B_LOC, KT, P, S).transpose(0, 2, 1, 3))
        xTU_t = np.ascontiguousarray(
            xTU.reshape(B_LOC, KT, P, UP).transpose(0, 2, 1, 3))
        in_maps.append({
            "xT": xT_t, "xTU": xTU_t,
            "wq0": wq0, "wqr": wqr, "wkf": wkf, "wvf": wvf,
            "bqp": bqp, "bkp": bkp, "bvr": bvr, "mkb": mkb,
        })

    nc = build(UP)
    res = run_bass_kernel_spmd(nc, in_maps, core_ids=list(range(NCORES)))
    att = np.concatenate([res.results[c]["att"] for c in range(NCORES)], axis=0)
    attw = np.zeros((B, S, S), dtype=np.float32)
    for c in range(NCORES):
        awc = res.results[c]["attwc"]                    # [B_LOC, S, UP]
        for bl in range(B_LOC):
            gb = c * B_LOC + bl
            attw[gb][:, idxs[gb]] = awc[bl][:, :ucounts[gb]]
    return att, attw
